# revision 16
# baseline (speedup 1.0000x reference)
"""Child-Sum TreeLSTM over complete binary trees — Trainium2 Bass kernel (v5.1).

Sharding: data-parallel over trees, B=32 across 8 NeuronCores (BL=4/core);
gate weights replicated.

Design:
  - bit-reversed node-major column layout per level: children of parent
    column p sit at columns p and p+HALF of the child level, so every
    child-pair op (h1+h2, f1*c1, f2*c2) is a packed contiguous DVE op
  - h stored fp8 only (resident h8 tile); the kernel outputs the
    (o, tanh(c)) pair in bf16 and the host multiplies them into h
  - h-side matmuls in fp8 DoubleRow (weights pre-scaled x16, descaled in
    the activation via scale=1/16); hs8 = h8_even + h8_odd pre-summed on
    DVE; dual-fp8 ldweights needs M%32==0 so m-chunks are (128,128,64)
    over a zero-padded M=320
  - f-gate x-side in fp8 DoubleRow from a resident ex8 tile (internal
    node columns only); i/o/u x-side stays bf16 for accuracy
  - f-gate split into even/odd-child passes (packed, no dup-broadcast)
  - h8 = o*th runs on GpSimd for large levels (DVE relief); output DMAs
    issue from SP; deep levels (nb<=128) merge i+o into one psum tile and
    one sigmoid activation
"""

import numpy as np
import ml_dtypes

import concourse.bass as bass
import concourse.mybir as mybir
import concourse.tile as tile
from concourse.ap import AP
from concourse import bacc
from concourse.bass_utils import run_bass_kernel_spmd

F32 = mybir.dt.float32
BF16 = mybir.dt.bfloat16
F8 = mybir.dt.float8e4
AF = mybir.ActivationFunctionType
DR = mybir.MatmulPerfMode.DoubleRow

B, D, DIM = 32, 11, 300
N = 2**D - 1          # 2047
CORES = 8
BL = B // CORES       # trees per core
NB = 512
CT = BL * N           # 8188
WS = 16.0
KO = [0, 128, 256]
MS = [128, 128, 64]   # output m-chunks padded: dual-fp8 ldweights needs M%32==0
MDIM = 320
PROJ = {"i": 0, "u": 1, "o": 2, "f": 3}
GF = {"i": AF.Sigmoid, "o": AF.Sigmoid, "u": AF.Tanh}


def _cols(l):
    return BL << l


LOFF = {}
_off = 0
for _l in range(D - 1, -1, -1):
    LOFF[_l] = _off
    _off += _cols(_l)

DEEP_L = 7
DOFF = LOFF[DEEP_L]           # 7168
DEEP_COLS = CT - DOFF         # 1020
IOFF = LOFF[D - 2]            # 4096: first internal column (level 9)
ICOLS = CT - IOFF             # 4092
MERGE_L = 5                   # levels <= this use merged psum/ACT layout

_NC_CACHE = []


def _pair_ap(t, part, chunk, col, nb, dstride):
    """AP [part, 2, nb] over tile t's chunk rows, dim-1 stride dstride
    (0 repeats the same rows — used as the moving side of a zero-weight
    DoubleRow pair slot)."""
    a = t[0:part, chunk, col:col + nb]
    return AP(a.tensor, a.offset, [[a.ap[0][0], part], [dstride, 2], [1, nb]])


def _build():
    nc = bacc.Bacc("TRN2", target_bir_lowering=False, debug=False,
                   num_devices=CORES)
    exT = nc.dram_tensor("ext", [128, 3, CT], BF16, kind="ExternalInput")
    ex8T = nc.dram_tensor("ex8t", [128, 3, ICOLS], F8, kind="ExternalInput")
    WX = nc.dram_tensor("wx", [128, 4, 3, MDIM], BF16, kind="ExternalInput")
    WH8 = nc.dram_tensor("wh8", [128, 4, 2, MDIM], F8, kind="ExternalInput")
    WH28 = nc.dram_tensor("wh28", [64, 4, 2, MDIM], F8, kind="ExternalInput")
    WH28B = nc.dram_tensor("wh28b", [64, 4, 2, MDIM], F8,
                           kind="ExternalInput")
    WFX8 = nc.dram_tensor("wfx8", [128, 2, MDIM], F8, kind="ExternalInput")
    WF2X8 = nc.dram_tensor("wf2x8", [64, 2, MDIM], F8, kind="ExternalInput")
    ooutT = nc.dram_tensor("oout", [128, 3, CT], BF16, kind="ExternalOutput")
    toutT = nc.dram_tensor("tout", [128, 3, CT], BF16, kind="ExternalOutput")

    with tile.TileContext(nc) as tc:
        import contextlib
        with contextlib.ExitStack() as ctx:
            sb = ctx.enter_context(tc.tile_pool(name="sb", bufs=1))
            exp = ctx.enter_context(tc.tile_pool(name="exp", bufs=6))
            gp = ctx.enter_context(tc.tile_pool(name="gp", bufs=14))
            fcp = ctx.enter_context(tc.tile_pool(name="fcp", bufs=4))
            psum = ctx.enter_context(
                tc.tile_pool(name="psum", bufs=2, space="PSUM"))

            wx_t = sb.tile([128, 4, 3, MDIM], BF16, name="wx_t")
            wh8_t = sb.tile([128, 4, 2, MDIM], F8, name="wh8_t")
            wh28_t = sb.tile([64, 4, 2, MDIM], F8, name="wh28_t")
            wh28b_t = sb.tile([64, 4, 2, MDIM], F8, name="wh28b_t")
            wfx8_t = sb.tile([128, 2, MDIM], F8, name="wfx8_t")
            wf2x8_t = sb.tile([64, 2, MDIM], F8, name="wf2x8_t")
            h8 = sb.tile([128, 3, CT], F8, name="h8")
            stc = sb.tile([128, 3, CT], BF16, name="stc")
            ex8 = sb.tile([128, 3, ICOLS], F8, name="ex8")
            exd = sb.tile([128, 3, DEEP_COLS], BF16, name="exd")

            # i,u weights first so the first leaf's matmuls start early
            nc.sync.dma_start(out=wx_t[:, 0:2, :, :], in_=WX[:, 0:2, :, :])
            setup2_done = []

            setup_steps = [
                lambda: nc.sync.dma_start(out=wx_t[:, 2:4, :, :],
                                          in_=WX[:, 2:4, :, :]),
                lambda: (nc.sync.dma_start(out=wh8_t[:, :, :, :],
                                           in_=WH8[:, :, :, :]),
                         nc.sync.dma_start(out=wh28_t[:, :, :, :],
                                           in_=WH28[:, :, :, :]),
                         nc.sync.dma_start(out=wh28b_t[:, :, :, :],
                                           in_=WH28B[:, :, :, :]),
                         nc.sync.dma_start(out=wfx8_t[:, :, :],
                                           in_=WFX8[:, :, :]),
                         nc.sync.dma_start(out=wf2x8_t[:, :, :],
                                           in_=WF2X8[:, :, :])),
                lambda: nc.sync.dma_start(out=exd[:, :, :],
                                          in_=exT[:, :, DOFF:CT]),
                lambda: nc.sync.dma_start(out=ex8[:, :, :],
                                          in_=ex8T[:, :, :]),
            ]

            def setup2():
                if setup_steps:
                    setup_steps.pop(0)()
                setup2_done.append(1)

            def mm_x(pG, po, nm, ex, e0, nb, stop):
                """bf16 x-side passes (k0, k1, k2+bias) for one gate.
                po: dim-1 base offset in the psum tile."""
                g_i = PROJ[nm]
                for m in range(3):
                    ms, mo = MS[m], KO[m]
                    for kk in (0, 1):
                        nc.tensor.matmul(
                            pG[0:ms, po + m, 0:nb],
                            wx_t[0:128, g_i, kk, mo:mo + ms],
                            ex[0:128, kk, e0:e0 + nb],
                            start=(kk == 0), stop=False)
                    nc.tensor.matmul(
                        pG[0:ms, po + m, 0:nb],
                        wx_t[0:45, g_i, 2, mo:mo + ms],
                        ex[0:45, 2, e0:e0 + nb],
                        start=False, stop=stop)

            def mm_h(pG, po, nm, cbE, cbO, nb):
                """fp8 DR h-side: per-child k01 pairs + h2 even|odd pair."""
                g_i = PROJ[nm]
                for m in range(3):
                    ms, mo = MS[m], KO[m]
                    nc.tensor.matmul(
                        pG[0:ms, po + m, 0:nb],
                        wh8_t[0:128, g_i, 0:2, mo:mo + ms],
                        h8[0:128, 0:2, cbE:cbE + nb],
                        perf_mode=DR, start=False, stop=False)
                    nc.tensor.matmul(
                        pG[0:ms, po + m, 0:nb],
                        wh8_t[0:128, g_i, 0:2, mo:mo + ms],
                        h8[0:128, 0:2, cbO:cbO + nb],
                        perf_mode=DR, start=False, stop=False)
                    nc.tensor.matmul(
                        pG[0:ms, po + m, 0:nb],
                        wh28b_t[0:44, g_i, 0:2, mo:mo + ms],
                        _pair_ap(h8, 44, 2, cbE, nb, cbO - cbE),
                        perf_mode=DR, start=False, stop=True)

            def mm_f(pF, po, cb, e8, nb):
                """f-gate: x-side fp8 DR from ex8 + per-child h fp8 DR."""
                for m in range(3):
                    ms, mo = MS[m], KO[m]
                    nc.tensor.matmul(
                        pF[0:ms, po + m, 0:nb],
                        wfx8_t[0:128, 0:2, mo:mo + ms],
                        ex8[0:128, 0:2, e8:e8 + nb],
                        perf_mode=DR, start=True, stop=False)
                    nc.tensor.matmul(
                        pF[0:ms, po + m, 0:nb],
                        wf2x8_t[0:45, 0:2, mo:mo + ms],
                        _pair_ap(ex8, 45, 2, e8, nb, 0),
                        perf_mode=DR, start=False, stop=False)
                    nc.tensor.matmul(
                        pF[0:ms, po + m, 0:nb],
                        wh8_t[0:128, PROJ["f"], 0:2, mo:mo + ms],
                        h8[0:128, 0:2, cb:cb + nb],
                        perf_mode=DR, start=False, stop=False)
                    nc.tensor.matmul(
                        pF[0:ms, po + m, 0:nb],
                        wh28_t[0:44, PROJ["f"], 0:2, mo:mo + ms],
                        _pair_ap(h8, 44, 2, cb, nb, 0),
                        perf_mode=DR, start=False, stop=True)

            # ---------------- leaf block (level 10) ----------------
            def leaf_block(k):
                c0 = k * NB
                o = LOFF[10] + c0
                ex = exp.tile([128, 3, NB], BF16, tag="ex", name=f"lex{k}")
                nc.sync.dma_start(out=ex[:, :, :], in_=exT[:, :, o:o + NB])
                setup2()
                sg = {}
                for nm in ("i", "u", "o"):
                    pG = psum.tile([128, 3, NB], F32, tag="pg",
                                   name=f"lpg{k}{nm}")
                    mm_x(pG, 0, nm, ex, 0, NB, stop=True)
                    g = gp.tile([128, 3, NB], BF16, tag="g", name=f"lg{k}{nm}")
                    nc.scalar.activation(g[:, :, :], pG[:, :, :], GF[nm],
                                         scale=1.0 / WS)
                    sg[nm] = g
                    if nm == "u":
                        cc = stc[:, :, o:o + NB]
                        nc.vector.tensor_mul(cc, sg["i"][:, :, :],
                                             sg["u"][:, :, :])
                        th = gp.tile([128, 3, NB], BF16, tag="g",
                                     name=f"lth{k}")
                        nc.scalar.activation(th[:, :, :], cc, AF.Tanh)
                nc.sync.dma_start(out=ooutT[:, :, o:o + NB],
                                  in_=sg["o"][:, :, :])
                nc.sync.dma_start(out=toutT[:, :, o:o + NB], in_=th[:, :, :])
                nc.gpsimd.tensor_mul(h8[:, :, o:o + NB],
                                     sg["o"][:, :, :], th[:, :, :])

            # ---------------- internal block (nb > 128) ----------------
            def block(l, c0, nb):
                half = _cols(l)
                o = LOFF[l] + c0
                cbE = LOFF[l + 1] + c0
                cbO = cbE + half
                if l >= 8:
                    ex = exp.tile([128, 3, NB], BF16, tag="ex",
                                  name=f"ex{l}_{c0}")
                    nc.sync.dma_start(out=ex[:, :, 0:nb],
                                      in_=exT[:, :, o:o + nb])
                    e0 = 0
                else:
                    ex = exd
                    e0 = o - DOFF
                e8 = o - IOFF

                pg = {}
                sg = {}
                for nm in ("i", "u"):
                    pg[nm] = psum.tile([128, 3, NB], F32, tag="pg",
                                       name=f"pg{l}_{c0}{nm}")
                    mm_x(pg[nm], 0, nm, ex, e0, nb, stop=False)
                for nm in ("i", "u"):
                    mm_h(pg[nm], 0, nm, cbE, cbO, nb)
                    g = gp.tile([128, 3, NB], BF16, tag="g",
                                name=f"g{l}_{c0}{nm}")
                    nc.scalar.activation(g[:, :, 0:nb], pg[nm][:, :, 0:nb],
                                         GF[nm], scale=1.0 / WS)
                    sg[nm] = g
                cc = stc[:, :, o:o + nb]
                nc.vector.tensor_mul(cc, sg["i"][:, :, 0:nb],
                                     sg["u"][:, :, 0:nb])

                def f_unit(side, cb):
                    pF = psum.tile([128, 3, NB], F32, tag="pg",
                                   name=f"pf{l}_{c0}{side}")
                    mm_f(pF, 0, cb, e8, nb)
                    f = gp.tile([128, 3, NB], BF16, tag="g",
                                name=f"f{l}_{c0}{side}")
                    nc.scalar.activation(f[:, :, 0:nb], pF[:, :, 0:nb],
                                         AF.Sigmoid, scale=1.0 / WS)
                    fc = fcp.tile([128, 3, NB], BF16, tag="fc",
                                  name=f"fc{l}_{c0}{side}")
                    nc.vector.tensor_mul(fc[:, :, 0:nb], f[:, :, 0:nb],
                                         stc[:, :, cb:cb + nb])
                    nc.vector.tensor_add(cc, cc, fc[:, :, 0:nb])

                f_unit("e", cbE)
                f_unit("o", cbO)
                pg["o"] = psum.tile([128, 3, NB], F32, tag="pg",
                                    name=f"pg{l}_{c0}o")
                mm_x(pg["o"], 0, "o", ex, e0, nb, stop=False)
                mm_h(pg["o"], 0, "o", cbE, cbO, nb)
                go = gp.tile([128, 3, NB], BF16, tag="g", name=f"g{l}_{c0}o")
                nc.scalar.activation(go[:, :, 0:nb], pg["o"][:, :, 0:nb],
                                     AF.Sigmoid, scale=1.0 / WS)

                th = gp.tile([128, 3, NB], BF16, tag="g", name=f"th{l}_{c0}")
                nc.scalar.activation(th[:, :, 0:nb], cc, AF.Tanh)
                nc.sync.dma_start(out=ooutT[:, :, o:o + nb],
                                  in_=go[:, :, 0:nb])
                nc.sync.dma_start(out=toutT[:, :, o:o + nb],
                                  in_=th[:, :, 0:nb])
                if l > 0:
                    eng = nc.gpsimd if nb >= 512 else nc.vector
                    eng.tensor_mul(h8[:, :, o:o + nb], go[:, :, 0:nb],
                                   th[:, :, 0:nb])

            # ---------------- schedule ----------------
            # wavefront: each level in quarters ordered (Q1,Q3,Q2,Q4) so a
            # level's first quarter unlocks after half of its child level
            for k in (0, 4, 2, 6):
                leaf_block(k)
            block(9, 0 * NB, NB)
            leaf_block(1)
            leaf_block(5)
            block(9, 2 * NB, NB)
            leaf_block(3)
            leaf_block(7)
            block(9, 1 * NB, NB)
            block(9, 3 * NB, NB)
            q8 = _cols(8) // 4
            for qi in (0, 2, 1, 3):
                block(8, qi * q8, q8)
            h7 = _cols(7) // 2
            block(7, 0, h7)
            block(7, h7, h7)
            for l in range(6, -1, -1):
                block(l, 0, _cols(l))
    nc.compile()
    return nc


def _revperm(l):
    n = 1 << l
    r = np.zeros(n, np.int64)
    for j in range(n):
        x, y = j, 0
        for _ in range(l):
            y = (y << 1) | (x & 1)
            x >>= 1
        r[j] = y
    return r


_REV = {l: _revperm(l) for l in range(D)}


def _pack_feats(dst, blk, col0, ncols):
    """blk: [DIM, ncols] float32 -> dst [128, 3, *] chunks at col0."""
    dst[0:128, 0, col0:col0 + ncols] = blk[0:128]
    dst[0:128, 1, col0:col0 + ncols] = blk[128:256]
    dst[0:44, 2, col0:col0 + ncols] = blk[256:300]


def kernel(embs, Wix, bix, Wih, bih, Wfx, bfx, Wfh, bfh,
           Wox, box, Woh, boh, Wux, bux, Wuh, buh):
    embs = np.ascontiguousarray(np.asarray(embs, dtype=np.float32))
    if not _NC_CACHE:
        _NC_CACHE.append(_build())
    nc = _NC_CACHE[0]
    bf = ml_dtypes.bfloat16
    f8 = ml_dtypes.float8_e4m3fn

    xw = {nm: np.asarray(w, np.float32)
          for nm, w in zip("ifou", (Wix, Wfx, Wox, Wux))}
    hw_ = {nm: np.asarray(w, np.float32)
           for nm, w in zip("ifou", (Wih, Wfh, Woh, Wuh))}
    xb = {"i": np.asarray(bix) + np.asarray(bih),
          "f": np.asarray(bfx) + np.asarray(bfh),
          "o": np.asarray(box) + np.asarray(boh),
          "u": np.asarray(bux) + np.asarray(buh)}

    wxp = np.zeros((128, 4, 3, MDIM), np.float32)
    wh8p = np.zeros((128, 4, 2, MDIM), np.float32)
    wh28p = np.zeros((64, 4, 2, MDIM), np.float32)
    wh28bp = np.zeros((64, 4, 2, MDIM), np.float32)
    wfx8p = np.zeros((128, 2, MDIM), np.float32)
    wf2x8p = np.zeros((64, 2, MDIM), np.float32)
    for nm, p in PROJ.items():
        w16 = xw[nm] * WS
        wxp[0:128, p, 0, 0:DIM] = w16[0:128]
        wxp[0:128, p, 1, 0:DIM] = w16[128:256]
        wxp[0:44, p, 2, 0:DIM] = w16[256:300]
        wxp[44, p, 2, 0:DIM] = xb[nm] * WS
        h16 = hw_[nm] * WS
        wh8p[0:128, p, 0, 0:DIM] = h16[0:128]
        wh8p[0:128, p, 1, 0:DIM] = h16[128:256]
        wh28p[0:44, p, 0, 0:DIM] = h16[256:300]
        wh28bp[0:44, p, 0, 0:DIM] = h16[256:300]
        wh28bp[0:44, p, 1, 0:DIM] = h16[256:300]
    wfx16 = xw["f"] * WS
    wfx8p[0:128, 0, 0:DIM] = wfx16[0:128]
    wfx8p[0:128, 1, 0:DIM] = wfx16[128:256]
    wf2x8p[0:44, 0, 0:DIM] = wfx16[256:300]
    wf2x8p[44, 0, 0:DIM] = xb["f"] * WS

    wxp = wxp.astype(bf)
    wh8p = wh8p.astype(f8)
    wh28p = wh28p.astype(f8)
    wh28bp = wh28bp.astype(f8)
    wfx8p = wfx8p.astype(f8)
    wf2x8p = wf2x8p.astype(f8)

    in_maps = []
    for c in range(CORES):
        e = embs[c * BL:(c + 1) * BL]
        exT = np.zeros((128, 3, CT), np.float32)
        ex8Tl = np.zeros((128, 3, ICOLS), np.float32)
        for l in range(D):
            base = (1 << l) - 1
            nl = 1 << l
            # [BL, nl, DIM] -> bit-reversed node-major [DIM, nl*BL]
            blk = e[:, base:base + nl, :][:, _REV[l], :]
            blk = blk.transpose(2, 1, 0).reshape(DIM, nl * BL)
            _pack_feats(exT, blk, LOFF[l], nl * BL)
            if l <= D - 2:
                _pack_feats(ex8Tl, blk, LOFF[l] - IOFF, nl * BL)
        exT[44, 2, :] = 1.0
        ex8Tl[44, 2, :] = 1.0
        in_maps.append({"ext": exT.astype(bf), "ex8t": ex8Tl.astype(f8),
                        "wx": wxp, "wh8": wh8p, "wh28": wh28p, "wh28b": wh28bp,
                        "wfx8": wfx8p, "wf2x8": wf2x8p})

    res = run_bass_kernel_spmd(nc, in_maps, list(range(CORES)))

    out = np.zeros((B, N, DIM), np.float32)
    for c in range(CORES):
        oT = np.asarray(res.results[c]["oout"]).astype(np.float32)
        tT = np.asarray(res.results[c]["tout"]).astype(np.float32)
        hT = oT * tT
        for l in range(D):
            base = (1 << l) - 1
            nl = 1 << l
            o = LOFF[l]
            feat = np.concatenate([hT[0:128, 0, o:o + nl * BL],
                                   hT[0:128, 1, o:o + nl * BL],
                                   hT[0:44, 2, o:o + nl * BL]], axis=0)
            # [DIM, nl, BL] -> [BL, nl, DIM], then undo bit-reversal
            arr = feat.reshape(DIM, nl, BL).transpose(2, 1, 0)
            out[c * BL:(c + 1) * BL, base:base + nl, :] = arr[:, _REV[l], :]
    return out


# revision 17
# speedup vs baseline: 1.1080x; 1.1080x over previous
"""Child-Sum TreeLSTM over complete binary trees — Trainium2 Bass kernel (v5.1).

Sharding: data-parallel over trees, B=32 across 8 NeuronCores (BL=4/core);
gate weights replicated.

Design:
  - bit-reversed node-major column layout per level: children of parent
    column p sit at columns p and p+HALF of the child level, so every
    child-pair op (h1+h2, f1*c1, f2*c2) is a packed contiguous DVE op
  - h stored fp8 only (resident h8 tile); the kernel outputs the
    (o, tanh(c)) pair in bf16 and the host multiplies them into h
  - h-side matmuls in fp8 DoubleRow (weights pre-scaled x16, descaled in
    the activation via scale=1/16); hs8 = h8_even + h8_odd pre-summed on
    DVE; dual-fp8 ldweights needs M%32==0 so m-chunks are (128,128,64)
    over a zero-padded M=320
  - f-gate x-side in fp8 DoubleRow from a resident ex8 tile (internal
    node columns only); i/o/u x-side stays bf16 for accuracy
  - f-gate split into even/odd-child passes (packed, no dup-broadcast)
  - h8 = o*th runs on GpSimd for large levels (DVE relief); output DMAs
    issue from SP; deep levels (nb<=128) merge i+o into one psum tile and
    one sigmoid activation
"""

import numpy as np
import ml_dtypes

import concourse.bass as bass
import concourse.mybir as mybir
import concourse.tile as tile
from concourse.ap import AP
from concourse import bacc
from concourse.bass_utils import run_bass_kernel_spmd

F32 = mybir.dt.float32
BF16 = mybir.dt.bfloat16
F8 = mybir.dt.float8e4
AF = mybir.ActivationFunctionType
DR = mybir.MatmulPerfMode.DoubleRow

B, D, DIM = 32, 11, 300
N = 2**D - 1          # 2047
CORES = 8
BL = B // CORES       # trees per core
NB = 512
CT = BL * N           # 8188
WS = 16.0
KO = [0, 128, 256]
MS = [128, 128, 64]   # output m-chunks padded: dual-fp8 ldweights needs M%32==0
MDIM = 320
PROJ = {"i": 0, "u": 1, "o": 2, "f": 3}
GF = {"i": AF.Sigmoid, "o": AF.Sigmoid, "u": AF.Tanh}


def _cols(l):
    return BL << l


LOFF = {}
_off = 0
for _l in range(D - 1, -1, -1):
    LOFF[_l] = _off
    _off += _cols(_l)

DEEP_L = 7
DOFF = LOFF[DEEP_L]           # 7168
DEEP_COLS = CT - DOFF         # 1020
IOFF = LOFF[D - 2]            # 4096: first internal column (level 9)
ICOLS = CT - IOFF             # 4092
MERGE_L = 5                   # levels <= this use merged psum/ACT layout

_NC_CACHE = []


def _pair_ap(t, part, chunk, col, nb, dstride):
    """AP [part, 2, nb] over tile t's chunk rows, dim-1 stride dstride
    (0 repeats the same rows — used as the moving side of a zero-weight
    DoubleRow pair slot)."""
    a = t[0:part, chunk, col:col + nb]
    return AP(a.tensor, a.offset, [[a.ap[0][0], part], [dstride, 2], [1, nb]])


def _build():
    nc = bacc.Bacc("TRN2", target_bir_lowering=False, debug=False,
                   num_devices=CORES)
    exT = nc.dram_tensor("ext", [128, 3, CT], BF16, kind="ExternalInput")
    ex8T = nc.dram_tensor("ex8t", [128, 3, ICOLS], F8, kind="ExternalInput")
    WX = nc.dram_tensor("wx", [128, 4, 3, MDIM], BF16, kind="ExternalInput")
    WH8 = nc.dram_tensor("wh8", [128, 4, 2, MDIM], F8, kind="ExternalInput")
    WH28 = nc.dram_tensor("wh28", [64, 4, 2, MDIM], F8, kind="ExternalInput")
    WH28B = nc.dram_tensor("wh28b", [64, 4, 2, MDIM], F8,
                           kind="ExternalInput")
    WFX8 = nc.dram_tensor("wfx8", [128, 2, MDIM], F8, kind="ExternalInput")
    WF2X8 = nc.dram_tensor("wf2x8", [64, 2, MDIM], F8, kind="ExternalInput")
    ooutT = nc.dram_tensor("oout", [128, 3, CT], BF16, kind="ExternalOutput")
    toutT = nc.dram_tensor("tout", [128, 3, CT], BF16, kind="ExternalOutput")

    with tile.TileContext(nc) as tc:
        import contextlib
        with contextlib.ExitStack() as ctx:
            sb = ctx.enter_context(tc.tile_pool(name="sb", bufs=1))
            exp = ctx.enter_context(tc.tile_pool(name="exp", bufs=6))
            gp = ctx.enter_context(tc.tile_pool(name="gp", bufs=14))
            fcp = ctx.enter_context(tc.tile_pool(name="fcp", bufs=4))
            hsp = ctx.enter_context(tc.tile_pool(name="hsp", bufs=3))
            psum = ctx.enter_context(
                tc.tile_pool(name="psum", bufs=2, space="PSUM"))

            wx_t = sb.tile([128, 4, 3, MDIM], BF16, name="wx_t")
            wh8_t = sb.tile([128, 4, 2, MDIM], F8, name="wh8_t")
            wh28_t = sb.tile([64, 4, 2, MDIM], F8, name="wh28_t")
            wh28b_t = sb.tile([64, 4, 2, MDIM], F8, name="wh28b_t")
            wfx8_t = sb.tile([128, 2, MDIM], F8, name="wfx8_t")
            wf2x8_t = sb.tile([64, 2, MDIM], F8, name="wf2x8_t")
            h8 = sb.tile([128, 3, CT], F8, name="h8")
            stc = sb.tile([128, 3, CT], BF16, name="stc")
            ex8 = sb.tile([128, 3, ICOLS], F8, name="ex8")
            exd = sb.tile([128, 3, DEEP_COLS], BF16, name="exd")

            # i,u weights first so the first leaf's matmuls start early
            nc.sync.dma_start(out=wx_t[:, 0:2, :, :], in_=WX[:, 0:2, :, :])
            setup2_done = []

            setup_steps = [
                lambda: nc.sync.dma_start(out=wx_t[:, 2:4, :, :],
                                          in_=WX[:, 2:4, :, :]),
                lambda: (nc.sync.dma_start(out=wh8_t[:, :, :, :],
                                           in_=WH8[:, :, :, :]),
                         nc.sync.dma_start(out=wh28_t[:, :, :, :],
                                           in_=WH28[:, :, :, :]),
                         nc.sync.dma_start(out=wh28b_t[:, :, :, :],
                                           in_=WH28B[:, :, :, :]),
                         nc.sync.dma_start(out=wfx8_t[:, :, :],
                                           in_=WFX8[:, :, :]),
                         nc.sync.dma_start(out=wf2x8_t[:, :, :],
                                           in_=WF2X8[:, :, :])),
                lambda: nc.sync.dma_start(out=exd[:, :, :],
                                          in_=exT[:, :, DOFF:CT]),
                lambda: nc.sync.dma_start(out=ex8[:, :, :],
                                          in_=ex8T[:, :, :]),
            ]

            def setup2():
                if setup_steps:
                    setup_steps.pop(0)()
                setup2_done.append(1)

            def mm_x(pG, po, nm, ex, e0, nb, stop):
                """bf16 x-side passes (k0, k1, k2+bias) for one gate.
                po: dim-1 base offset in the psum tile."""
                g_i = PROJ[nm]
                for m in range(3):
                    ms, mo = MS[m], KO[m]
                    for kk in (0, 1):
                        nc.tensor.matmul(
                            pG[0:ms, po + m, 0:nb],
                            wx_t[0:128, g_i, kk, mo:mo + ms],
                            ex[0:128, kk, e0:e0 + nb],
                            start=(kk == 0), stop=False)
                    nc.tensor.matmul(
                        pG[0:ms, po + m, 0:nb],
                        wx_t[0:45, g_i, 2, mo:mo + ms],
                        ex[0:45, 2, e0:e0 + nb],
                        start=False, stop=stop)

            def mm_h_pc(pG, po, nm, cbE, cbO, nb):
                """fp8 DR h-side: per-child k01 pairs + h2 even|odd pair."""
                g_i = PROJ[nm]
                for m in range(3):
                    ms, mo = MS[m], KO[m]
                    nc.tensor.matmul(
                        pG[0:ms, po + m, 0:nb],
                        wh8_t[0:128, g_i, 0:2, mo:mo + ms],
                        h8[0:128, 0:2, cbE:cbE + nb],
                        perf_mode=DR, start=False, stop=False)
                    nc.tensor.matmul(
                        pG[0:ms, po + m, 0:nb],
                        wh8_t[0:128, g_i, 0:2, mo:mo + ms],
                        h8[0:128, 0:2, cbO:cbO + nb],
                        perf_mode=DR, start=False, stop=False)
                    nc.tensor.matmul(
                        pG[0:ms, po + m, 0:nb],
                        wh28b_t[0:44, g_i, 0:2, mo:mo + ms],
                        _pair_ap(h8, 44, 2, cbE, nb, cbO - cbE),
                        perf_mode=DR, start=False, stop=True)

            def mm_h_hs(pG, po, nm, hs8, nb):
                """fp8 DR h-side from pre-summed hs8 (k01 pair, k2+zero)."""
                g_i = PROJ[nm]
                for m in range(3):
                    ms, mo = MS[m], KO[m]
                    nc.tensor.matmul(
                        pG[0:ms, po + m, 0:nb],
                        wh8_t[0:128, g_i, 0:2, mo:mo + ms],
                        hs8[0:128, 0:2, 0:nb],
                        perf_mode=DR, start=False, stop=False)
                    nc.tensor.matmul(
                        pG[0:ms, po + m, 0:nb],
                        wh28_t[0:44, g_i, 0:2, mo:mo + ms],
                        _pair_ap(hs8, 44, 2, 0, nb, 0),
                        perf_mode=DR, start=False, stop=True)

            def mm_f(pF, po, cb, e8, nb):
                """f-gate: x-side fp8 DR from ex8 + per-child h fp8 DR."""
                for m in range(3):
                    ms, mo = MS[m], KO[m]
                    nc.tensor.matmul(
                        pF[0:ms, po + m, 0:nb],
                        wfx8_t[0:128, 0:2, mo:mo + ms],
                        ex8[0:128, 0:2, e8:e8 + nb],
                        perf_mode=DR, start=True, stop=False)
                    nc.tensor.matmul(
                        pF[0:ms, po + m, 0:nb],
                        wf2x8_t[0:45, 0:2, mo:mo + ms],
                        _pair_ap(ex8, 45, 2, e8, nb, 0),
                        perf_mode=DR, start=False, stop=False)
                    nc.tensor.matmul(
                        pF[0:ms, po + m, 0:nb],
                        wh8_t[0:128, PROJ["f"], 0:2, mo:mo + ms],
                        h8[0:128, 0:2, cb:cb + nb],
                        perf_mode=DR, start=False, stop=False)
                    nc.tensor.matmul(
                        pF[0:ms, po + m, 0:nb],
                        wh28_t[0:44, PROJ["f"], 0:2, mo:mo + ms],
                        _pair_ap(h8, 44, 2, cb, nb, 0),
                        perf_mode=DR, start=False, stop=True)

            # ---------------- leaf block (level 10) ----------------
            def leaf_block(k):
                c0 = k * NB
                o = LOFF[10] + c0
                ex = exp.tile([128, 3, NB], BF16, tag="ex", name=f"lex{k}")
                nc.sync.dma_start(out=ex[:, :, :], in_=exT[:, :, o:o + NB])
                setup2()
                sg = {}
                for nm in ("i", "u", "o"):
                    pG = psum.tile([128, 3, NB], F32, tag="pg",
                                   name=f"lpg{k}{nm}")
                    mm_x(pG, 0, nm, ex, 0, NB, stop=True)
                    g = gp.tile([128, 3, NB], BF16, tag="g", name=f"lg{k}{nm}")
                    nc.scalar.activation(g[:, :, :], pG[:, :, :], GF[nm],
                                         scale=1.0 / WS)
                    sg[nm] = g
                    if nm == "u":
                        cc = stc[:, :, o:o + NB]
                        nc.vector.tensor_mul(cc, sg["i"][:, :, :],
                                             sg["u"][:, :, :])
                        th = gp.tile([128, 3, NB], BF16, tag="g",
                                     name=f"lth{k}")
                        nc.scalar.activation(th[:, :, :], cc, AF.Tanh)
                nc.sync.dma_start(out=ooutT[:, :, o:o + NB],
                                  in_=sg["o"][:, :, :])
                nc.sync.dma_start(out=toutT[:, :, o:o + NB], in_=th[:, :, :])
                nc.gpsimd.tensor_mul(h8[:, :, o:o + NB],
                                     sg["o"][:, :, :], th[:, :, :])

            # ---------------- internal block (nb > 128) ----------------
            def block(l, c0, nb):
                half = _cols(l)
                o = LOFF[l] + c0
                cbE = LOFF[l + 1] + c0
                cbO = cbE + half
                if l >= 8:
                    ex = exp.tile([128, 3, NB], BF16, tag="ex",
                                  name=f"ex{l}_{c0}")
                    nc.sync.dma_start(out=ex[:, :, 0:nb],
                                      in_=exT[:, :, o:o + nb])
                    e0 = 0
                else:
                    ex = exd
                    e0 = o - DOFF
                e8 = o - IOFF

                use_hs = nb >= 256
                if use_hs:
                    hs8 = hsp.tile([128, 3, NB], F8, tag="hs",
                                   name=f"hs{l}_{c0}")
                    nc.vector.tensor_add(hs8[:, 0:2, 0:nb],
                                         h8[:, 0:2, cbE:cbE + nb],
                                         h8[:, 0:2, cbO:cbO + nb])
                    nc.vector.tensor_add(hs8[0:44, 2, 0:nb],
                                         h8[0:44, 2, cbE:cbE + nb],
                                         h8[0:44, 2, cbO:cbO + nb])

                def mm_h(pG, po, nm):
                    if use_hs:
                        mm_h_hs(pG, po, nm, hs8, nb)
                    else:
                        mm_h_pc(pG, po, nm, cbE, cbO, nb)

                pg = {}
                sg = {}
                for nm in ("i", "u"):
                    pg[nm] = psum.tile([128, 3, NB], F32, tag="pg",
                                       name=f"pg{l}_{c0}{nm}")
                    mm_x(pg[nm], 0, nm, ex, e0, nb, stop=False)
                for nm in ("i", "u"):
                    mm_h(pg[nm], 0, nm)
                    g = gp.tile([128, 3, NB], BF16, tag="g",
                                name=f"g{l}_{c0}{nm}")
                    nc.scalar.activation(g[:, :, 0:nb], pg[nm][:, :, 0:nb],
                                         GF[nm], scale=1.0 / WS)
                    sg[nm] = g
                cc = stc[:, :, o:o + nb]
                nc.vector.tensor_mul(cc, sg["i"][:, :, 0:nb],
                                     sg["u"][:, :, 0:nb])

                def f_unit(side, cb):
                    pF = psum.tile([128, 3, NB], F32, tag="pg",
                                   name=f"pf{l}_{c0}{side}")
                    mm_f(pF, 0, cb, e8, nb)
                    f = gp.tile([128, 3, NB], BF16, tag="g",
                                name=f"f{l}_{c0}{side}")
                    nc.scalar.activation(f[:, :, 0:nb], pF[:, :, 0:nb],
                                         AF.Sigmoid, scale=1.0 / WS)
                    fc = fcp.tile([128, 3, NB], BF16, tag="fc",
                                  name=f"fc{l}_{c0}{side}")
                    nc.vector.tensor_mul(fc[:, :, 0:nb], f[:, :, 0:nb],
                                         stc[:, :, cb:cb + nb])
                    nc.vector.tensor_add(cc, cc, fc[:, :, 0:nb])

                f_unit("e", cbE)
                f_unit("o", cbO)
                pg["o"] = psum.tile([128, 3, NB], F32, tag="pg",
                                    name=f"pg{l}_{c0}o")
                mm_x(pg["o"], 0, "o", ex, e0, nb, stop=False)
                mm_h(pg["o"], 0, "o")
                go = gp.tile([128, 3, NB], BF16, tag="g", name=f"g{l}_{c0}o")
                nc.scalar.activation(go[:, :, 0:nb], pg["o"][:, :, 0:nb],
                                     AF.Sigmoid, scale=1.0 / WS)

                th = gp.tile([128, 3, NB], BF16, tag="g", name=f"th{l}_{c0}")
                nc.scalar.activation(th[:, :, 0:nb], cc, AF.Tanh)
                nc.sync.dma_start(out=ooutT[:, :, o:o + nb],
                                  in_=go[:, :, 0:nb])
                nc.sync.dma_start(out=toutT[:, :, o:o + nb],
                                  in_=th[:, :, 0:nb])
                if l > 0:
                    eng = nc.gpsimd if nb >= 512 else nc.vector
                    eng.tensor_mul(h8[:, :, o:o + nb], go[:, :, 0:nb],
                                   th[:, :, 0:nb])

            # ---------------- schedule ----------------
            # wavefront: each level in quarters ordered (Q1,Q3,Q2,Q4) so a
            # level's first quarter unlocks after half of its child level
            for k in (0, 4, 2, 6):
                leaf_block(k)
            block(9, 0 * NB, NB)
            leaf_block(1)
            leaf_block(5)
            block(9, 2 * NB, NB)
            leaf_block(3)
            leaf_block(7)
            block(9, 1 * NB, NB)
            block(9, 3 * NB, NB)
            q8 = _cols(8) // 4
            for qi in (0, 2, 1, 3):
                block(8, qi * q8, q8)
            h7 = _cols(7) // 2
            block(7, 0, h7)
            block(7, h7, h7)
            for l in range(6, -1, -1):
                block(l, 0, _cols(l))
    nc.compile()
    return nc


def _revperm(l):
    n = 1 << l
    r = np.zeros(n, np.int64)
    for j in range(n):
        x, y = j, 0
        for _ in range(l):
            y = (y << 1) | (x & 1)
            x >>= 1
        r[j] = y
    return r


_REV = {l: _revperm(l) for l in range(D)}


def _pack_feats(dst, blk, col0, ncols):
    """blk: [DIM, ncols] float32 -> dst [128, 3, *] chunks at col0."""
    dst[0:128, 0, col0:col0 + ncols] = blk[0:128]
    dst[0:128, 1, col0:col0 + ncols] = blk[128:256]
    dst[0:44, 2, col0:col0 + ncols] = blk[256:300]


def kernel(embs, Wix, bix, Wih, bih, Wfx, bfx, Wfh, bfh,
           Wox, box, Woh, boh, Wux, bux, Wuh, buh):
    embs = np.ascontiguousarray(np.asarray(embs, dtype=np.float32))
    if not _NC_CACHE:
        _NC_CACHE.append(_build())
    nc = _NC_CACHE[0]
    bf = ml_dtypes.bfloat16
    f8 = ml_dtypes.float8_e4m3fn

    xw = {nm: np.asarray(w, np.float32)
          for nm, w in zip("ifou", (Wix, Wfx, Wox, Wux))}
    hw_ = {nm: np.asarray(w, np.float32)
           for nm, w in zip("ifou", (Wih, Wfh, Woh, Wuh))}
    xb = {"i": np.asarray(bix) + np.asarray(bih),
          "f": np.asarray(bfx) + np.asarray(bfh),
          "o": np.asarray(box) + np.asarray(boh),
          "u": np.asarray(bux) + np.asarray(buh)}

    wxp = np.zeros((128, 4, 3, MDIM), np.float32)
    wh8p = np.zeros((128, 4, 2, MDIM), np.float32)
    wh28p = np.zeros((64, 4, 2, MDIM), np.float32)
    wh28bp = np.zeros((64, 4, 2, MDIM), np.float32)
    wfx8p = np.zeros((128, 2, MDIM), np.float32)
    wf2x8p = np.zeros((64, 2, MDIM), np.float32)
    for nm, p in PROJ.items():
        w16 = xw[nm] * WS
        wxp[0:128, p, 0, 0:DIM] = w16[0:128]
        wxp[0:128, p, 1, 0:DIM] = w16[128:256]
        wxp[0:44, p, 2, 0:DIM] = w16[256:300]
        wxp[44, p, 2, 0:DIM] = xb[nm] * WS
        h16 = hw_[nm] * WS
        wh8p[0:128, p, 0, 0:DIM] = h16[0:128]
        wh8p[0:128, p, 1, 0:DIM] = h16[128:256]
        wh28p[0:44, p, 0, 0:DIM] = h16[256:300]
        wh28bp[0:44, p, 0, 0:DIM] = h16[256:300]
        wh28bp[0:44, p, 1, 0:DIM] = h16[256:300]
    wfx16 = xw["f"] * WS
    wfx8p[0:128, 0, 0:DIM] = wfx16[0:128]
    wfx8p[0:128, 1, 0:DIM] = wfx16[128:256]
    wf2x8p[0:44, 0, 0:DIM] = wfx16[256:300]
    wf2x8p[44, 0, 0:DIM] = xb["f"] * WS

    wxp = wxp.astype(bf)
    wh8p = wh8p.astype(f8)
    wh28p = wh28p.astype(f8)
    wh28bp = wh28bp.astype(f8)
    wfx8p = wfx8p.astype(f8)
    wf2x8p = wf2x8p.astype(f8)

    in_maps = []
    for c in range(CORES):
        e = embs[c * BL:(c + 1) * BL]
        exT = np.zeros((128, 3, CT), np.float32)
        ex8Tl = np.zeros((128, 3, ICOLS), np.float32)
        for l in range(D):
            base = (1 << l) - 1
            nl = 1 << l
            # [BL, nl, DIM] -> bit-reversed node-major [DIM, nl*BL]
            blk = e[:, base:base + nl, :][:, _REV[l], :]
            blk = blk.transpose(2, 1, 0).reshape(DIM, nl * BL)
            _pack_feats(exT, blk, LOFF[l], nl * BL)
            if l <= D - 2:
                _pack_feats(ex8Tl, blk, LOFF[l] - IOFF, nl * BL)
        exT[44, 2, :] = 1.0
        ex8Tl[44, 2, :] = 1.0
        in_maps.append({"ext": exT.astype(bf), "ex8t": ex8Tl.astype(f8),
                        "wx": wxp, "wh8": wh8p, "wh28": wh28p, "wh28b": wh28bp,
                        "wfx8": wfx8p, "wf2x8": wf2x8p})

    res = run_bass_kernel_spmd(nc, in_maps, list(range(CORES)))

    out = np.zeros((B, N, DIM), np.float32)
    for c in range(CORES):
        oT = np.asarray(res.results[c]["oout"]).astype(np.float32)
        tT = np.asarray(res.results[c]["tout"]).astype(np.float32)
        hT = oT * tT
        for l in range(D):
            base = (1 << l) - 1
            nl = 1 << l
            o = LOFF[l]
            feat = np.concatenate([hT[0:128, 0, o:o + nl * BL],
                                   hT[0:128, 1, o:o + nl * BL],
                                   hT[0:44, 2, o:o + nl * BL]], axis=0)
            # [DIM, nl, BL] -> [BL, nl, DIM], then undo bit-reversal
            arr = feat.reshape(DIM, nl, BL).transpose(2, 1, 0)
            out[c * BL:(c + 1) * BL, base:base + nl, :] = arr[:, _REV[l], :]
    return out
